# revision 5
# baseline (speedup 1.0000x reference)
"""Trainium2 Bass kernel for nn_CooccurrenceMatrix (V3 — minimal RPC traffic).

Reference computation (per batch b, walks r/s in [0,W), positions i/j in [0,L)):
    match[b,r,s,i,j] = (a[b,r,i] == a[b,s,j]) & mask[b,r,i] & mask[b,s,j]
    C[b,r,s]  = sum_{i,j} match * K[i,j]
    valid[b,w] = sum_i mask[b,w,i]
    out = C / (valid[:,r]*valid[:,s] + 1e-8)

Device computes the RAW weighted co-occurrence C only; the exact
normalization (valid outer product + eps) is applied on host in f64.
Since C[b] is symmetric, the two per-core batches are packed into one
[W, W+1] bf16 tile: upper triangle (incl diag) = batch 0, strict lower
triangle = batch 1, extra column = diag of batch 1.

Algorithm used on device (per batch):
    One-hot features FT[(v,i), w] = (a[w,i]+1 == v+1 masked)   (400 feats)
    GT = kron(I5, K^T) @ FT   (Gaussian kernel along i, per value v)
    C  = FT^T @ GT  accumulated over 4 feature chunks of 100

Sharding: pure data-parallel, batch dim 16 -> 2 batches on each of 8 cores.
"""

import os
import tempfile

import numpy as np
import ml_dtypes

B, W, L = 16, 128, 20
NCORES = 8
BL = B // NCORES          # batches per core (2)
V = L                     # number of distinct node values (20)
NV = 5                    # v-values per feature chunk
NCHUNK = V // NV          # 4 chunks
KF = NV * L               # features per chunk (100)
FREE = BL * W             # packed free dim (256)

_CACHE = {}


def _config_jax_cache():
    """Enable jax's persistent compilation cache so repeated calls skip the
    XLA->NEFF compile path (run_bass_kernel_spmd builds a fresh closure per
    call, so the in-memory jit cache can never hit)."""
    try:
        import jax

        cache_dir = os.path.join(tempfile.gettempdir(), "jax_bass_cache")
        for key, val in (
            ("jax_compilation_cache_dir", cache_dir),
            ("jax_persistent_cache_min_compile_time_secs", 0.0),
            ("jax_persistent_cache_min_entry_size_bytes", -1),
        ):
            try:
                jax.config.update(key, val)
            except Exception:
                pass
    except Exception:
        pass


_config_jax_cache()


def _split_drain_waits(nc, maxw=1):
    """Workaround: this container's walrus rejects instructions carrying more
    than ~1 semaphore wait ("Too many sync wait commands" in setupSyncWait).
    Move excess waits onto chained same-engine NOPs directly before the
    instruction — semantically identical, the engine just stalls stepwise."""
    import concourse.mybir as mybir

    for f in nc.m.functions:
        for blk in f.blocks:
            insts = list(blk.instructions)
            out = []
            changed = False
            for ins in insts:
                si = ins.sync_info
                if si is not None and len(si.on_wait) > maxw:
                    waits = list(si.on_wait)
                    k = 0
                    while len(waits) > maxw:
                        chunk, waits = waits[:maxw], waits[maxw:]
                        nop = mybir.InstNoOp(name=f"{ins.name}-ws{k}", ins=[], outs=[])
                        nop.engine = ins.engine
                        nop.sync_info = mybir.SyncInfo(on_wait=chunk, on_update=[])
                        out.append(nop)
                        k += 1
                    ins.sync_info = mybir.SyncInfo(
                        on_wait=waits, on_update=list(si.on_update)
                    )
                    changed = True
                out.append(ins)
            if changed:
                blk.instructions = out
    return nc


def _build_nc():
    import concourse.bass as bass
    import concourse.mybir as mybir
    import concourse.tile as tile
    from concourse.masks import make_identity, make_upper_triangular

    bf16 = mybir.dt.bfloat16
    f32 = mybir.dt.float32

    nc = bass.Bass("TRN2")

    # single packed input: cols 0:40 masked codes (both batches),
    # cols 40:44 rows 0:100 = per-chunk compare values,
    # cols 44:144 rows 0:100 = kron(I5, K^T)
    PK = BL * L + NCHUNK + KF
    x_d = nc.dram_tensor("xin", [W, PK], bf16, kind="ExternalInput")
    out_d = nc.dram_tensor("out", [W, W + 1], bf16, kind="ExternalOutput")

    with tile.TileContext(nc) as tc:
        with (
            tc.tile_pool(name="sb", bufs=1) as sb,
            tc.tile_pool(name="ps", bufs=1, space="PSUM") as ps,
        ):
            ident = sb.tile([W, W], bf16)
            make_identity(nc, ident[:])
            tri = sb.tile([W, W], mybir.dt.int8)
            make_upper_triangular(nc, tri[:], val=1.0, diag=True)

            pk = sb.tile([W, PK], bf16)
            nc.sync.dma_start(out=pk[:], in_=x_d[:])
            vvf = sb.tile([KF, NCHUNK], f32)
            nc.scalar.copy(out=vvf[:], in_=pk[0:KF, BL * L : BL * L + NCHUNK])

            # Replicate masked codes 5x along the free dim (DVE broadcast
            # copy), then PE-transpose so the replication lands on (v,i)
            # partitions.
            xrep = sb.tile([W, BL * KF], bf16)
            for b in range(BL):
                nc.vector.tensor_copy(
                    out=xrep[:, b * KF : (b + 1) * KF].rearrange(
                        "p (v i) -> p v i", v=NV
                    ),
                    in_=pk[:, b * L : (b + 1) * L]
                    .rearrange("p (o i) -> p o i", o=1)
                    .to_broadcast([W, NV, L]),
                )
            psumT = ps.tile([KF, FREE], bf16)
            for b in range(BL):
                nc.tensor.transpose(
                    out=psumT[:, b * W : (b + 1) * W],
                    in_=xrep[:, b * KF : (b + 1) * KF],
                    identity=ident[:],
                )

            # one-hot chunks + Gaussian-kernel matmuls
            ft = []
            gt = []
            for c in range(NCHUNK):
                ftc = sb.tile([KF, FREE], bf16, name=f"ft{c}", tag=f"ft{c}")
                nc.vector.tensor_scalar(
                    out=ftc[:],
                    in0=psumT[:],
                    scalar1=vvf[:, c : c + 1],
                    scalar2=None,
                    op0=mybir.AluOpType.is_equal,
                )
                ft.append(ftc)
            for half in range(2):
                gpsum = ps.tile([KF, 2 * FREE], f32, name=f"gp{half}", tag=f"gp{half}")
                for ci in range(2):
                    c = half * 2 + ci
                    nc.tensor.matmul(
                        out=gpsum[:, ci * FREE : (ci + 1) * FREE],
                        lhsT=pk[0:KF, BL * L + NCHUNK : PK],
                        rhs=ft[c][:],
                        start=True,
                        stop=True,
                    )
                for ci in range(2):
                    c = half * 2 + ci
                    gtc = sb.tile([KF, FREE], bf16, name=f"gt{c}", tag=f"gt{c}")
                    nc.scalar.copy(
                        out=gtc[:], in_=gpsum[:, ci * FREE : (ci + 1) * FREE]
                    )
                    gt.append(gtc)

            # raw co-occurrence accumulation, per batch
            cps = []
            for b in range(BL):
                cp = ps.tile([W, W], f32, name=f"cp{b}", tag=f"cp{b}")
                for c in range(NCHUNK):
                    nc.tensor.matmul(
                        out=cp[:],
                        lhsT=ft[c][:, b * W : (b + 1) * W],
                        rhs=gt[c][:, b * W : (b + 1) * W],
                        start=(c == 0),
                        stop=(c == NCHUNK - 1),
                    )
                cps.append(cp)

            # pack: upper(incl diag) = batch0, strict lower = batch1,
            # extra column = diag of batch1
            outsb = sb.tile([W, W + 1], bf16)
            nc.vector.select(
                out=outsb[:, 0:W], mask=tri[:], on_true=cps[0][:], on_false=cps[1][:]
            )
            dtmp = sb.tile([W, W], bf16)
            nc.vector.tensor_tensor(
                out=dtmp[:], in0=cps[1][:], in1=ident[:], op=mybir.AluOpType.mult
            )
            with nc.allow_low_precision(reason="one nonzero per row (diag pick)"):
                nc.vector.tensor_reduce(
                    out=outsb[:, W : W + 1],
                    in_=dtmp[:],
                    axis=mybir.AxisListType.X,
                    op=mybir.AluOpType.add,
                )

            nc.sync.dma_start(out=out_d[:], in_=outsb[:])

    return nc


def _host_consts(K):
    bf16 = ml_dtypes.bfloat16
    p = np.arange(KF)
    vv = np.empty((KF, NCHUNK), dtype=np.float32)
    for c in range(NCHUNK):
        vv[:, c] = (NV * c + p // L) + 1.0
    mblk = np.kron(np.eye(NV, dtype=np.float32), K.T.astype(np.float32))
    return vv.astype(np.float32), mblk.astype(bf16)


def _prepare(inputs):
    bf16 = ml_dtypes.bfloat16
    a = np.asarray(inputs["anonymized_nodes"]).astype(np.int64)   # [B, W, L]
    m = np.asarray(inputs["walk_masks"]).astype(np.float32)       # [B, W, L]
    K = np.asarray(inputs["kernel"]).astype(np.float32)           # [L, L]

    if "nc" not in _CACHE:
        _CACHE["nc"] = _split_drain_waits(_build_nc())
    nc = _CACHE["nc"]

    vv, mblk = _host_consts(K)

    # masked codes: a+1 where mask>0 else 0 (0 never matches any v+1)
    am = np.where(m > 0, a + 1, 0).astype(bf16)                    # [B, W, L]

    PK = BL * L + NCHUNK + KF
    in_maps = []
    for ci in range(NCORES):
        loc = am[ci * BL : (ci + 1) * BL]                          # [BL, W, L]
        x = np.zeros((W, PK), dtype=bf16)
        x[:, 0 : BL * L] = loc.transpose(1, 0, 2).reshape(W, BL * L)
        x[0:KF, BL * L : BL * L + NCHUNK] = vv.astype(bf16)
        x[0:KF, BL * L + NCHUNK : PK] = mblk
        in_maps.append({"xin": x})

    valid = m.sum(axis=-1, dtype=np.float64)                       # [B, W]
    return nc, in_maps, valid


_UPPER = np.triu(np.ones((W, W), dtype=bool))          # incl diag
_STRICT_UPPER = np.triu(np.ones((W, W), dtype=bool), 1)


def _gather(results, valid):
    pk = np.stack([np.asarray(results[ci]["out"]) for ci in range(NCORES)])
    pk = pk.astype(np.float32)                                     # [NC, W, W+1]
    sq, d1 = pk[:, :, :W], pk[:, :, W]
    out = np.empty((B, W, W), dtype=np.float32)
    c0 = out[0::2]
    c1 = out[1::2]
    # batch0: upper incl diag, mirrored; batch1: strict lower, mirrored + diag col
    np.multiply(sq, _UPPER, out=c0)
    c0 += np.where(_STRICT_UPPER, sq, 0.0).transpose(0, 2, 1)
    np.multiply(sq, ~_UPPER, out=c1)
    c1 += np.where(~_UPPER, sq, 0.0).transpose(0, 2, 1)
    c1[:, np.arange(W), np.arange(W)] = d1
    norm = (valid[:, :, None] * valid[:, None, :] + 1e-8).astype(np.float32)
    out /= norm
    return out


def kernel(**inputs):
    nc, in_maps, valid = _prepare(inputs)

    from concourse.bass_utils import run_bass_kernel_spmd

    res = run_bass_kernel_spmd(nc, in_maps, core_ids=list(range(NCORES)))
    return _gather(res.results, valid)
